# revision 6
# baseline (speedup 1.0000x reference)
"""AnomalyAttention Trainium2 kernel (8 NeuronCores, SPMD data-parallel over batch).

Math (per b,h):
  series = softmax(causal_mask(Q K^T / 8))          = E / sum(E)
  prior  = rownorm(exp(-(l-s)^2 / (2 sigma'^2)))    = G / sum(G)   (banded: |l-s|<=16 matters)
  fused  = g*series + (1-g)*prior ; renormalize     (sum == 1 -> renorm skipped, err ~1e-6)
  out    = fused @ V = a*(E@V) + b*(G@V),  a = g/sum(E), b = (1-g)/sum(G)  per row.

Implementation choices:
  - bf16 matmuls (tolerance 2e-2), scores computed from xbar-DMA-transposed Q/K.
  - causal mask added in PSUM via identity-matmul of a -240 upper-tri constant.
  - exp on ScalarE with fused per-partition scale (AP) + accum_out row sums.
  - Gaussian prior computed only on a 160-wide band window per 128-row chunk.
  - normalization postponed to after the PV matmul (tiny [128,64] tiles).
"""

import math
from contextlib import ExitStack

import ml_dtypes
import numpy as np

import concourse.bass as bass
import concourse.mybir as mybir
import concourse.tile as tile
from concourse import bacc
from concourse.bass_utils import run_bass_kernel_spmd

F32 = mybir.dt.float32
BF16 = mybir.dt.bfloat16
AF = mybir.ActivationFunctionType
OP = mybir.AluOpType

B, L, H, E = 16, 512, 8, 64
NCORES = 8
BPC = B // NCORES  # batches per core
PC = 128           # partition chunk
NCH = L // PC      # 4 chunks of 128 rows
BAND = 160         # gaussian band window width (s in [128*li-16, 128*li+144))
BOFF = 16
MASKVAL = -240.0   # exp(0.125*(s-240)) <= e^-24 ~ 0
LN3 = math.log(3.0)

_CACHE = {}
LAST_RESULT = None


def _consts():
    ident = np.eye(PC, dtype=ml_dtypes.bfloat16)
    mtri = np.triu(np.full((PC, PC), MASKVAL, dtype=np.float32), k=1).astype(
        ml_dtypes.bfloat16
    )
    # dist2 variants [3, 128, BAND]: d2[p, j] = (j - 16 - p)^2 ; poisoned out-of-range
    p = np.arange(PC)[:, None]
    j = np.arange(BAND)[None, :]
    d2 = ((j - BOFF - p) ** 2).astype(np.float32)
    d2_first = d2.copy()
    d2_first[:, :BOFF] = 1e30  # li=0: s = j-16 < 0 invalid
    d2_last = d2.copy()
    d2_last[:, 144:] = 1e30    # li=3: s = 368+j+... >= 512 invalid (j >= 144)
    dist2 = np.stack([d2_first, d2, d2_last])  # var 0,1,2
    ones = np.ones((1, PC), dtype=np.float32)
    return ident, mtri, dist2, ones


def _build():
    if "nc" in _CACHE:
        return _CACHE["nc"]
    nc = bacc.Bacc()
    ident_np, mtri_np, dist2_np, ones_np = _consts()

    q_h = nc.dram_tensor("queries", [BPC, L, H, E], F32, kind="ExternalInput")
    k_h = nc.dram_tensor("keys", [BPC, L, H, E], F32, kind="ExternalInput")
    v_h = nc.dram_tensor("values", [BPC, L, H, E], F32, kind="ExternalInput")
    sig_h = nc.dram_tensor("sigma", [BPC, L, H], F32, kind="ExternalInput")
    hgl_h = nc.dram_tensor("hgl", [1, H], F32, kind="ExternalInput")
    out_h = nc.dram_tensor("out", [BPC, L, H, E], F32, kind="ExternalOutput")

    ident_d = nc.inline_tensor(ident_np, name="identc")
    mtri_d = nc.inline_tensor(mtri_np, name="mtric")
    dist2_d = nc.inline_tensor(dist2_np, name="dist2c")
    ones_d = nc.inline_tensor(ones_np, name="onesc")

    with ExitStack() as ctx:
        tc = ctx.enter_context(tile.TileContext(nc))
        const = ctx.enter_context(tc.tile_pool(name="const", bufs=1))
        qkT = ctx.enter_context(tc.tile_pool(name="qkT", bufs=2))
        vpool = ctx.enter_context(tc.tile_pool(name="vpool", bufs=2))
        spool = ctx.enter_context(tc.tile_pool(name="spool", bufs=2))
        epool = ctx.enter_context(tc.tile_pool(name="epool", bufs=3))
        etpool = ctx.enter_context(tc.tile_pool(name="etpool", bufs=2))
        gpool = ctx.enter_context(tc.tile_pool(name="gpool", bufs=3))
        gtpool = ctx.enter_context(tc.tile_pool(name="gtpool", bufs=2))
        small = ctx.enter_context(tc.tile_pool(name="small", bufs=3))
        outp = ctx.enter_context(tc.tile_pool(name="outp", bufs=2))
        tmpp = ctx.enter_context(tc.tile_pool(name="tmpp", bufs=4))
        ps_s = ctx.enter_context(tc.tile_pool(name="ps_s", bufs=2, space="PSUM"))
        ps_t = ctx.enter_context(tc.tile_pool(name="ps_t", bufs=2, space="PSUM"))
        ps_gt = ctx.enter_context(tc.tile_pool(name="ps_gt", bufs=2, space="PSUM"))
        ps_u = ctx.enter_context(tc.tile_pool(name="ps_u", bufs=1, space="PSUM"))
        dram = ctx.enter_context(tc.tile_pool(name="dram", bufs=2, space="DRAM"))

        # ---- constants into SBUF ----
        ident = const.tile([PC, PC], BF16, tag="ident")
        nc.sync.dma_start(ident, ident_d[:, :])
        mtri = const.tile([PC, PC], BF16, tag="mtri")
        nc.sync.dma_start(mtri, mtri_d[:, :])
        d2sb = const.tile([PC, 3 * BAND], F32, tag="d2sb")
        for v in range(3):
            nc.sync.dma_start(d2sb[:, v * BAND:(v + 1) * BAND], dist2_d[v, :, :])
        ones_sb = const.tile([1, PC], F32, tag="ones")
        nc.sync.dma_start(ones_sb, ones_d[:, :])

        # ---- gates (once per core) ----
        hgl_sb = const.tile([1, H], F32, tag="hgl")
        nc.sync.dma_start(hgl_sb, hgl_h[:, :])
        ge = const.tile([1, H], F32, tag="ge")
        nc.scalar.activation(ge, hgl_sb, AF.Exp, scale=-1.0)  # exp(-x)
        gp = const.tile([1, H], F32, tag="gp")
        nc.vector.tensor_scalar_add(gp, ge, 1.0)
        gate = const.tile([1, H], F32, tag="gate")
        nc.vector.reciprocal(gate, gp)  # sigmoid(x)
        gb_ps = ps_s.tile([PC, H], F32, tag="S")
        nc.tensor.matmul(gb_ps, ones_sb, gate, start=True, stop=True)
        gates_b = const.tile([PC, H], F32, tag="gatesb")
        nc.vector.tensor_copy(gates_b, gb_ps)
        omg_b = const.tile([PC, H], F32, tag="omgb")
        nc.vector.tensor_scalar(omg_b, gates_b, -1.0, 1.0, OP.mult, OP.add)

        for bi in range(BPC):
            # ---- Q/K: load f32, convert bf16, transpose via PE (per batch) ----
            Qb = []
            Kb = []
            for t in range(4):
                qf = tmpp.tile([PC, H * E], F32, tag=f"qf{t}")
                nc.sync.dma_start(qf, q_h[bi, t * PC:(t + 1) * PC, :, :])
                qb = tmpp.tile([PC, H * E], BF16, tag=f"qb{t}")
                nc.vector.tensor_copy(qb, qf)
                Qb.append(qb)
                kf = tmpp.tile([PC, H * E], F32, tag=f"kf{t}")
                nc.sync.dma_start(kf, k_h[bi, t * PC:(t + 1) * PC, :, :])
                kb = tmpp.tile([PC, H * E], BF16, tag=f"kb{t}")
                nc.vector.tensor_copy(kb, kf)
                Kb.append(kb)
            QT = []
            KT = []
            for to in range(4):
                ps_q = ps_t.tile([PC, L], BF16, tag="ETp")
                for ti in range(4):
                    nc.tensor.transpose(
                        ps_q[:, ti * PC:(ti + 1) * PC],
                        Qb[ti][:, to * PC:(to + 1) * PC], ident,
                    )
                qt = qkT.tile([PC, L], BF16, tag=f"qT{to}")
                nc.vector.tensor_copy(qt, ps_q)
                QT.append(qt)
                ps_k = ps_t.tile([PC, L], BF16, tag="ETp")
                for ti in range(4):
                    nc.tensor.transpose(
                        ps_k[:, ti * PC:(ti + 1) * PC],
                        Kb[ti][:, to * PC:(to + 1) * PC], ident,
                    )
                kt = qkT.tile([PC, L], BF16, tag=f"kT{to}")
                nc.vector.tensor_copy(kt, ps_k)
                KT.append(kt)

            # ---- V natural (load f32 + DVE cast) + shifted-by-16 copies ----
            Vn = []
            for t in range(4):
                vf = tmpp.tile([PC, H * E], F32, tag=f"vf{t}")
                nc.sync.dma_start(vf, v_h[bi, t * PC:(t + 1) * PC, :, :])
                vt = vpool.tile([PC, H * E], BF16, tag=f"vn{t}")
                nc.vector.tensor_copy(vt, vf)
                Vn.append(vt)
            Vs = []
            for t in range(5):
                vs = vpool.tile([PC, H * E], BF16, tag=f"vs{t}")
                if t == 0:
                    # rows [0,16) zero, rows [16,128) <- V rows [0,112)
                    nc.gpsimd.memset(vs[0:32, :], 0.0)
                    nc.sync.dma_start(vs[BOFF:PC, :], Vn[0][0:PC - BOFF, :])
                elif t == 4:
                    # only rows [0,32) read: [0,16) = V tail, [16,32) = zero
                    nc.gpsimd.memset(vs[0:32, :], 0.0)
                    nc.sync.dma_start(vs[0:BOFF, :], Vn[3][PC - BOFF:PC, :])
                else:
                    nc.sync.dma_start(vs[0:BOFF, :], Vn[t - 1][PC - BOFF:PC, :])
                    nc.sync.dma_start(vs[BOFF:PC, :], Vn[t][0:PC - BOFF, :])
                Vs.append(vs)

            # ---- sigma -> m = -0.5 / sigma'^2, laid out [128, 4*H] col = 8*li+h ----
            sraw = spool.tile([PC, NCH * H], F32, tag="sraw")
            for c in range(NCH):
                nc.sync.dma_start(
                    sraw[:, c * H:(c + 1) * H], sig_h[bi, c * PC:(c + 1) * PC, :]
                )
            e5 = spool.tile([PC, NCH * H], F32, tag="e5")
            nc.scalar.activation(e5, sraw, AF.Exp, scale=-5.0)
            p1 = spool.tile([PC, NCH * H], F32, tag="p1")
            nc.vector.tensor_scalar_add(p1, e5, 1.0)
            sg = spool.tile([PC, NCH * H], F32, tag="sg")
            nc.vector.reciprocal(sg, p1)  # sigmoid(5x)
            sg2 = spool.tile([PC, NCH * H], F32, tag="sg2")
            nc.vector.tensor_scalar_add(sg2, sg, 1e-5)
            p3 = spool.tile([PC, NCH * H], F32, tag="p3")
            nc.scalar.activation(p3, sg2, AF.Exp, scale=LN3)
            sm1 = spool.tile([PC, NCH * H], F32, tag="sm1")
            nc.vector.tensor_scalar_add(sm1, p3, -1.0)  # sigma'
            s2 = spool.tile([PC, NCH * H], F32, tag="s2")
            nc.vector.tensor_tensor(s2, sm1, sm1, OP.mult)
            r2 = spool.tile([PC, NCH * H], F32, tag="r2")
            nc.vector.reciprocal(r2, s2)
            mm = spool.tile([PC, NCH * H], F32, tag="mm")
            nc.vector.tensor_scalar_mul(mm, r2, -0.5)

            oslab = []
            for li in range(NCH):
                ot = outp.tile([PC, H * E], F32, tag=f"o{li}")
                oslab.append(ot)

            for h in range(H):
                t4 = h // 2
                po = 64 * (h % 2)
                vc = 64 * h
                sume = small.tile([PC, NCH], F32, tag="sume")
                sumg = small.tile([PC, NCH], F32, tag="sumg")
                ETs = []
                GT_ps = ps_gt.tile([PC, 1024], BF16, tag="gtps")
                for li in range(NCH):
                    S = ps_s.tile([PC, L], F32, tag="S")
                    lhQ = QT[t4][po:po + 64, li * PC:(li + 1) * PC]
                    if li > 0:
                        nc.tensor.matmul(
                            S[:, 0:li * PC], lhQ, KT[t4][po:po + 64, 0:li * PC],
                            start=True, stop=True,
                        )
                    nc.tensor.matmul(
                        S[:, li * PC:(li + 1) * PC], lhQ,
                        KT[t4][po:po + 64, li * PC:(li + 1) * PC],
                        start=True, stop=False,
                    )
                    nc.tensor.matmul(
                        S[:, li * PC:(li + 1) * PC], ident, mtri,
                        start=False, stop=True,
                    )
                    ncz = PC * (li + 1)
                    Et = epool.tile([PC, L], BF16, tag="E")
                    nc.scalar.activation(
                        Et[:, :ncz], S[:, :ncz], AF.Exp, scale=0.125,
                        accum_out=sume[:, li:li + 1],
                    )
                    var = 0 if li == 0 else (2 if li == 3 else 1)
                    Gt = gpool.tile([PC, BAND], BF16, tag="G")
                    nc.scalar.activation(
                        Gt, d2sb[:, var * BAND:(var + 1) * BAND], AF.Exp,
                        scale=mm[:, li * H + h:li * H + h + 1],
                        accum_out=sumg[:, li:li + 1],
                    )
                    # transposes
                    ETp = ps_t.tile([PC, L], BF16, tag="ETp")
                    for sj in range(li + 1):
                        nc.tensor.transpose(
                            ETp[:, sj * PC:(sj + 1) * PC],
                            Et[:, sj * PC:(sj + 1) * PC], ident,
                        )
                    ET = etpool.tile([PC, L], BF16, tag=f"ET{li}")
                    nc.vector.tensor_copy(ET[:, :ncz], ETp[:, :ncz])
                    ETs.append(ET)
                    nc.tensor.transpose(GT_ps[:, li * PC:(li + 1) * PC], Gt[:, 0:PC], ident)
                    nc.tensor.transpose(
                        GT_ps[0:2 * BOFF, L + li * PC:L + (li + 1) * PC],
                        Gt[:, PC:BAND], ident,
                    )
                GT1 = gtpool.tile([PC, L], BF16, tag="GT1")
                nc.vector.tensor_copy(GT1, GT_ps[:, 0:L])
                GT2 = gtpool.tile([2 * BOFF, L], BF16, tag="GT2")
                nc.vector.tensor_copy(GT2, GT_ps[0:2 * BOFF, L:2 * L])

                re = small.tile([PC, NCH], F32, tag="re")
                nc.vector.reciprocal(re, sume)
                rg = small.tile([PC, NCH], F32, tag="rg")
                nc.vector.reciprocal(rg, sumg)
                av = small.tile([PC, NCH], F32, tag="av")
                nc.vector.tensor_scalar_mul(av, re, gates_b[:, h:h + 1])
                bv = small.tile([PC, NCH], F32, tag="bv")
                nc.vector.tensor_scalar_mul(bv, rg, omg_b[:, h:h + 1])

                U1 = ps_u.tile([PC, L], F32, tag="U1")
                U2 = ps_u.tile([PC, L], F32, tag="U2")
                for li in range(NCH):
                    for sj in range(li + 1):
                        nc.tensor.matmul(
                            U1[:, li * PC:li * PC + 64],
                            ETs[li][:, sj * PC:(sj + 1) * PC],
                            Vn[sj][:, vc:vc + 64],
                            start=(sj == 0), stop=(sj == li),
                        )
                    nc.tensor.matmul(
                        U2[:, li * PC:li * PC + 64],
                        GT1[:, li * PC:(li + 1) * PC],
                        Vs[li][:, vc:vc + 64],
                        start=True, stop=False,
                    )
                    nc.tensor.matmul(
                        U2[:, li * PC:li * PC + 64],
                        GT2[:, li * PC:(li + 1) * PC],
                        Vs[li + 1][0:2 * BOFF, vc:vc + 64],
                        start=False, stop=True,
                    )
                for li in range(NCH):
                    t2 = tmpp.tile([PC, 64], F32, tag="t2")
                    nc.scalar.activation(
                        t2, U2[:, li * PC:li * PC + 64], AF.Copy,
                        bias=0.0, scale=bv[:, li:li + 1],
                    )
                    nc.vector.scalar_tensor_tensor(
                        oslab[li][:, vc:vc + 64],
                        U1[:, li * PC:li * PC + 64],
                        av[:, li:li + 1], t2, OP.mult, OP.add,
                    )

            for li in range(NCH):
                nc.sync.dma_start(out_h[bi, li * PC:(li + 1) * PC, :, :], oslab[li])

    nc.compile()
    _CACHE["nc"] = nc
    return nc


def kernel(**inputs):
    global LAST_RESULT
    nc = _build()
    q = np.ascontiguousarray(inputs["queries"], dtype=np.float32)
    k = np.ascontiguousarray(inputs["keys"], dtype=np.float32)
    v = np.ascontiguousarray(inputs["values"], dtype=np.float32)
    sg = np.ascontiguousarray(inputs["sigma"], dtype=np.float32)
    hgl = np.ascontiguousarray(inputs["head_gate_logit"], dtype=np.float32).reshape(1, H)

    in_maps = []
    for c in range(NCORES):
        b0 = BPC * c
        in_maps.append({
            "queries": q[b0:b0 + BPC],
            "keys": k[b0:b0 + BPC],
            "values": v[b0:b0 + BPC],
            "sigma": sg[b0:b0 + BPC],
            "hgl": hgl,
        })
    res = run_bass_kernel_spmd(nc, in_maps, core_ids=list(range(NCORES)))
    LAST_RESULT = res
    out = np.concatenate([r["out"] for r in res.results], axis=0)
    return out.astype(np.float32)
